# Initial kernel scaffold
#
"""DynamicA8W8 MoE FFN on 8 TRN2 NeuronCores.

Sizes (hardcoded from the problem spec):
  T=4096 tokens, H=4096 hidden, I=1408 intermediate, E=16 experts,
  equal contiguous groups of TPE=256 tokens per expert.

Sharding: expert-parallel == token-parallel here (contiguous equal groups).
Core c owns experts {2c, 2c+1} and tokens [512c, 512c+512). No cross-core
communication is needed; each core computes its own [512, H] output slab and
the host concatenates.

Per-core pipeline:
  1. per-token dynamic quant of x -> int8 (RNE+saturate via f32->int8 copy),
     exact in bf16; PE-transpose to [h, t] layout for use as matmul stationary.
  2. grouped GEMM1 vs w13 (int8 weights DMA'd raw, cast to bf16 on chip;
     bf16 matmul is exact for int8 operands, fp32 PSUM accumulate).
  3. SwiGLU epilogue fused with dequant scales, dynamic requant to int8.
  4. GEMM2 vs w2, fused per-channel + per-token dequant, DMA out.
"""

import json

import numpy as np

import concourse.bass as bass
import concourse.bass2jax as bass2jax
import concourse.mybir as mybir
from concourse.bass_utils import run_bass_kernel_spmd
from concourse.masks import make_identity
from concourse.tile import TileContext

F32 = mybir.dt.float32
BF16 = mybir.dt.bfloat16
I8 = mybir.dt.int8
AF = mybir.ActivationFunctionType
ALU = mybir.AluOpType
AX = mybir.AxisListType

T, H, I, E = 4096, 4096, 1408, 16
NCORES = 8
E_LOC = E // NCORES            # 2 experts per core
TPE = T // E                   # 256 tokens per expert
T_LOC = E_LOC * TPE            # 512 tokens per core
NTB = T_LOC // 128             # 4 token blocks per core
KT1 = H // 128                 # 32 k-tiles for mm1
KT2 = I // 128                 # 11 k-tiles for mm2
# gate/up column chunks (free dim of mm1, <=512 per PSUM bank)
I_CHUNKS = [(0, 512), (512, 512), (1024, 384)]
H_CHUNKS = [(c, 512) for c in range(0, H, 512)]


# --- walrus workaround: this build rejects >1 sync wait per instruction.
# Split extras into standalone single-wait EventSemaphore instructions placed
# immediately before, on the same engine queue.
def _split_multi_waits(bir_json: bytes) -> bytes:
    j = json.loads(bir_json)
    changed = False
    for fn in j.get("functions", []):
        for blk in fn.get("blocks", []):
            out = []
            for inst in blk.get("instructions", []):
                si = inst.get("sync_info")
                waits = si.get("on_wait") if si else None
                if waits and len(waits) > 1:
                    spill, keep = waits[:-1], waits[-1:]
                    for k, w in enumerate(spill):
                        out.append({
                            "debug": inst.get("debug", 0),
                            "engine": inst["engine"],
                            "ins": [], "outs": [],
                            "name": f"{inst['name']}_w{k}",
                            "opcode": "EventSemaphore",
                            "sync_info": {"on_update": [], "on_wait": [w]},
                        })
                    si["on_wait"] = keep
                    changed = True
                out.append(inst)
            blk["instructions"] = out
    return json.dumps(j).encode() if changed else bir_json


_hook_installed = False


def _install_compile_hook():
    global _hook_installed
    if _hook_installed:
        return
    orig = bass2jax.compile_bir_kernel

    def wrapped(bir_json, tmpdir, neff_name="file.neff"):
        return orig(_split_multi_waits(bir_json), tmpdir, neff_name=neff_name)

    bass2jax.compile_bir_kernel = wrapped
    _hook_installed = True


def _cast_engine(nc, idx):
    """Round-robin the int8->bf16 weight casts across ACT/DVE/Pool."""
    r = idx % 8
    if r < 5:
        return nc.scalar.copy
    if r < 7:
        return nc.vector.tensor_copy
    return nc.gpsimd.tensor_copy


def _build_program():
    nc = bass.Bass()
    x_d = nc.declare_dram_parameter("x", [T_LOC, H], F32, isOutput=False)
    w13T_d = nc.declare_dram_parameter("w13T", [E_LOC, H, 2 * I], I8, isOutput=False)
    w2T_d = nc.declare_dram_parameter("w2T", [E_LOC, I, H], I8, isOutput=False)
    wsg_d = nc.declare_dram_parameter("wsg", [E_LOC, 128, I], F32, isOutput=False)
    wsu_d = nc.declare_dram_parameter("wsu", [E_LOC, 128, I], F32, isOutput=False)
    w2s_d = nc.declare_dram_parameter("w2s", [E_LOC, 128, H], F32, isOutput=False)
    out_d = nc.declare_dram_parameter("out", [T_LOC, H], F32, isOutput=True)

    with TileContext(nc) as tc:
        with (
            tc.tile_pool(name="const", bufs=1) as const,
            tc.tile_pool(name="xload", bufs=1) as xload,
            tc.tile_pool(name="xq", bufs=1) as xqp,
            tc.tile_pool(name="xqt", bufs=NTB) as xqtp,
            tc.tile_pool(name="small", bufs=NTB) as small,
            tc.tile_pool(name="wload", bufs=2) as wload,
            tc.tile_pool(name="wcast", bufs=4) as wcast,
            tc.tile_pool(name="scales", bufs=2) as scalep,
            tc.tile_pool(name="w2scale", bufs=1) as w2scalep,
            tc.tile_pool(name="hbuf", bufs=2) as hbuf,
            tc.tile_pool(name="hq", bufs=2) as hqp,
            tc.tile_pool(name="outp", bufs=4) as outp,
            tc.tile_pool(name="pt", bufs=2, space="PSUM") as ptp,
            tc.tile_pool(name="pg", bufs=2, space="PSUM") as pgp,
            tc.tile_pool(name="pu", bufs=2, space="PSUM") as pup,
            tc.tile_pool(name="p2", bufs=2, space="PSUM") as p2p,
        ):
            ident = const.tile([128, 128], BF16)
            make_identity(nc, ident)

            # ---- Phase 0: dynamic-quantize all token blocks, build xqT ----
            xqT = []     # per t-block: [128h, KT1, 128t] bf16
            s1s = []     # per t-block: [128, 1] f32 quant scale
            for tb in range(NTB):
                xt = xload.tile([128, H], F32)
                nc.sync.dma_start(xt[:], x_d[tb * 128:(tb + 1) * 128, :])
                amax = small.tile([128, 1], F32, tag="amax1")
                nc.vector.tensor_reduce(amax[:], xt[:], axis=AX.X, op=ALU.max,
                                        apply_absolute_value=True)
                s1 = small.tile([128, 1], F32, tag="s1")
                nc.vector.tensor_scalar(s1[:], amax[:], 1.0 / 127.0, None,
                                        op0=ALU.mult)
                inv1 = small.tile([128, 1], F32, tag="inv1")
                nc.vector.reciprocal(inv1[:], s1[:])
                xq_i8 = xqp.tile([128, H], I8, tag="xq_i8")
                nc.vector.tensor_scalar(xq_i8[:], xt[:], inv1[:], None,
                                        op0=ALU.mult)
                xq_bf = xqp.tile([128, H], BF16, tag="xq_bf")
                nc.scalar.copy(xq_bf[:], xq_i8[:])
                xqt = xqtp.tile([128, KT1, 128], BF16)
                for k in range(KT1):
                    pt = ptp.tile([128, 128], BF16)
                    nc.tensor.transpose(pt[:], xq_bf[:, k * 128:(k + 1) * 128],
                                        ident[:])
                    nc.scalar.copy(xqt[:, k, :], pt[:])
                xqT.append(xqt)
                s1s.append(s1)

            cast_idx = 0
            # ---- Expert loop ----
            for e in range(E_LOC):
                tbs = [2 * e, 2 * e + 1]
                wsg = scalep.tile([128, I], F32, tag="wsg")
                nc.sync.dma_start(wsg[:], wsg_d[e])
                wsu = scalep.tile([128, I], F32, tag="wsu")
                nc.sync.dma_start(wsu[:], wsu_d[e])

                # h accumulators for this expert's two token blocks
                hts = [hbuf.tile([128, I], F32) for _ in tbs]

                # ---- mm1 + SwiGLU epilogue, chunked over gate/up columns ----
                for (c0, cw) in I_CHUNKS:
                    wg_i8 = [wload.tile([128, KT1 // 2, cw], I8, tag="wg_i8")
                             for _ in range(2)]
                    wu_i8 = [wload.tile([128, KT1 // 2, cw], I8, tag="wu_i8")
                             for _ in range(2)]
                    g_src = w13T_d[e, :, c0:c0 + cw].rearrange(
                        "(k p) o -> p k o", p=128)
                    u_src = w13T_d[e, :, I + c0:I + c0 + cw].rearrange(
                        "(k p) o -> p k o", p=128)
                    for h2 in range(2):
                        ksl = slice(h2 * (KT1 // 2), (h2 + 1) * (KT1 // 2))
                        nc.sync.dma_start(wg_i8[h2][:], g_src[:, ksl, :])
                        nc.sync.dma_start(wu_i8[h2][:], u_src[:, ksl, :])

                    pg = [pgp.tile([128, cw], F32) for _ in tbs]
                    pu = [pup.tile([128, cw], F32) for _ in tbs]
                    for k in range(KT1):
                        h2, kk = divmod(k, KT1 // 2)
                        wg_bf = wcast.tile([128, cw], BF16, tag="wg_bf")
                        _cast_engine(nc, cast_idx)(wg_bf[:], wg_i8[h2][:, kk, :])
                        cast_idx += 1
                        wu_bf = wcast.tile([128, cw], BF16, tag="wu_bf")
                        _cast_engine(nc, cast_idx)(wu_bf[:], wu_i8[h2][:, kk, :])
                        cast_idx += 1
                        st = (k == 0)
                        sp = (k == KT1 - 1)
                        for i_tb, tb in enumerate(tbs):
                            nc.tensor.matmul(pg[i_tb][:], xqT[tb][:, k, :],
                                             wg_bf[:], start=st, stop=sp)
                            nc.tensor.matmul(pu[i_tb][:], xqT[tb][:, k, :],
                                             wu_bf[:], start=st, stop=sp)

                    for i_tb, tb in enumerate(tbs):
                        gate = outp.tile([128, cw], F32, tag="gate")
                        nc.vector.scalar_tensor_tensor(
                            gate[:], pg[i_tb][:], s1s[tb][:],
                            wsg[:, c0:c0 + cw], op0=ALU.mult, op1=ALU.mult)
                        up = outp.tile([128, cw], F32, tag="up")
                        nc.vector.scalar_tensor_tensor(
                            up[:], pu[i_tb][:], s1s[tb][:],
                            wsu[:, c0:c0 + cw], op0=ALU.mult, op1=ALU.mult)
                        sg = outp.tile([128, cw], F32, tag="sg")
                        nc.scalar.activation(sg[:], gate[:], AF.Silu)
                        nc.vector.tensor_mul(hts[i_tb][:, c0:c0 + cw],
                                             sg[:], up[:])

                # ---- dynamic requant of h, build hqT ----
                hqT = []
                s2s = []
                for i_tb, tb in enumerate(tbs):
                    amax2 = small.tile([128, 1], F32, tag="amax2")
                    nc.vector.tensor_reduce(amax2[:], hts[i_tb][:], axis=AX.X,
                                            op=ALU.max, apply_absolute_value=True)
                    s2 = small.tile([128, 1], F32, tag="s2")
                    nc.vector.tensor_scalar(s2[:], amax2[:], 1.0 / 127.0, None,
                                            op0=ALU.mult)
                    inv2 = small.tile([128, 1], F32, tag="inv2")
                    nc.vector.reciprocal(inv2[:], s2[:])
                    hq_i8 = hqp.tile([128, I], I8, tag="hq_i8")
                    nc.vector.tensor_scalar(hq_i8[:], hts[i_tb][:], inv2[:],
                                            None, op0=ALU.mult)
                    hq_bf = hqp.tile([128, I], BF16, tag="hq_bf")
                    nc.scalar.copy(hq_bf[:], hq_i8[:])
                    hqt = hqp.tile([128, KT2, 128], BF16, tag="hqT")
                    for k in range(KT2):
                        pt = ptp.tile([128, 128], BF16)
                        nc.tensor.transpose(pt[:], hq_bf[:, k * 128:(k + 1) * 128],
                                            ident[:])
                        nc.scalar.copy(hqt[:, k, :], pt[:])
                    hqT.append(hqt)
                    s2s.append(s2)

                # ---- mm2 + output dequant ----
                w2s = w2scalep.tile([128, H], F32, tag="w2s")
                nc.sync.dma_start(w2s[:], w2s_d[e])
                for (c0, cw) in H_CHUNKS:
                    w2_i8 = wload.tile([128, KT2, cw], I8, tag="w2_i8")
                    nc.sync.dma_start(
                        w2_i8[:],
                        w2T_d[e, :, c0:c0 + cw].rearrange("(k p) o -> p k o",
                                                          p=128))
                    p2 = [p2p.tile([128, cw], F32) for _ in tbs]
                    for k in range(KT2):
                        w2_bf = wcast.tile([128, cw], BF16, tag="w2_bf")
                        _cast_engine(nc, cast_idx)(w2_bf[:], w2_i8[:, k, :])
                        cast_idx += 1
                        for i_tb in range(2):
                            nc.tensor.matmul(p2[i_tb][:], hqT[i_tb][:, k, :],
                                             w2_bf[:], start=(k == 0),
                                             stop=(k == KT2 - 1))
                    for i_tb, tb in enumerate(tbs):
                        ot = outp.tile([128, cw], F32, tag="ot")
                        nc.vector.scalar_tensor_tensor(
                            ot[:], p2[i_tb][:], s2s[i_tb][:],
                            w2s[:, c0:c0 + cw], op0=ALU.mult, op1=ALU.mult)
                        nc.sync.dma_start(
                            out_d[tb * 128:(tb + 1) * 128, c0:c0 + cw], ot[:])

    return nc


_cached_nc = None


def kernel(x, w13, w2, w13_scale, smooth_scale_2, w2_scale, expert_tokens):
    global _cached_nc
    _install_compile_hook()
    # expert_tokens describes the fixed equal contiguous grouping (the
    # reference ignores it); we rely on that same grouping.
    del expert_tokens

    x = np.asarray(x, dtype=np.float32)
    w13 = np.asarray(w13).astype(np.int8, copy=False)
    w2 = np.asarray(w2).astype(np.int8, copy=False)
    w13_scale = np.asarray(w13_scale, dtype=np.float32)
    smooth_scale_2 = np.asarray(smooth_scale_2, dtype=np.float32)
    w2_scale = np.asarray(w2_scale, dtype=np.float32)

    # Fold the (linear) smooth scale into the up-projection dequant scale.
    wsu_full = w13_scale[:, I:] * smooth_scale_2          # [E, I]
    wsg_full = w13_scale[:, :I]                           # [E, I]

    in_maps = []
    for c in range(NCORES):
        es = slice(E_LOC * c, E_LOC * (c + 1))
        ts = slice(T_LOC * c, T_LOC * (c + 1))
        in_maps.append({
            "x": np.ascontiguousarray(x[ts]),
            "w13T": np.ascontiguousarray(w13[es].transpose(0, 2, 1)),
            "w2T": np.ascontiguousarray(w2[es].transpose(0, 2, 1)),
            "wsg": np.ascontiguousarray(
                np.broadcast_to(wsg_full[es, None, :], (E_LOC, 128, I))),
            "wsu": np.ascontiguousarray(
                np.broadcast_to(wsu_full[es, None, :], (E_LOC, 128, I))),
            "w2s": np.ascontiguousarray(
                np.broadcast_to(w2_scale[es, None, :], (E_LOC, 128, H))),
        })

    if _cached_nc is None:
        _cached_nc = _build_program()
    res = run_bass_kernel_spmd(_cached_nc, in_maps, list(range(NCORES)))
    return np.concatenate([res.results[c]["out"] for c in range(NCORES)],
                          axis=0)


# revision 13
# speedup vs baseline: 1.3063x; 1.3063x over previous
"""DynamicA8W8 MoE FFN on 8 TRN2 NeuronCores.

Sizes (hardcoded from the problem spec):
  T=4096 tokens, H=4096 hidden, I=1408 intermediate, E=16 experts,
  equal contiguous groups of TPE=256 tokens per expert.

Sharding: expert-parallel == token-parallel here (contiguous equal groups).
Core c owns experts {2c, 2c+1} and tokens [512c, 512c+512). No cross-core
communication is needed; each core computes its own [512, H] output slab and
the host concatenates.

Per-core pipeline:
  1. per-token dynamic quant of x -> int8 (RNE+saturate via f32->int8 copy),
     exact in bf16; PE-transpose to [h, t] layout for use as matmul stationary.
  2. grouped GEMM1 vs w13 (int8 weights DMA'd raw, cast to bf16 on chip;
     bf16 matmul is exact for int8 operands, fp32 PSUM accumulate).
  3. SwiGLU epilogue fused with dequant scales, dynamic requant to int8.
  4. GEMM2 vs w2, fused per-channel + per-token dequant, DMA out.
"""

import json

import numpy as np

import concourse.bass as bass
import concourse.bass2jax as bass2jax
import concourse.mybir as mybir
from concourse.bass_utils import run_bass_kernel_spmd
from concourse.masks import make_identity
from concourse.tile import TileContext

F32 = mybir.dt.float32
BF16 = mybir.dt.bfloat16
I8 = mybir.dt.int8
AF = mybir.ActivationFunctionType
ALU = mybir.AluOpType
AX = mybir.AxisListType

T, H, I, E = 4096, 4096, 1408, 16
NCORES = 8
E_LOC = E // NCORES            # 2 experts per core
TPE = T // E                   # 256 tokens per expert
T_LOC = E_LOC * TPE            # 512 tokens per core
NTB = T_LOC // 128             # 4 token blocks per core
KT1 = H // 128                 # 32 k-tiles for mm1
KT2 = I // 128                 # 11 k-tiles for mm2
# gate/up column chunks (free dim of mm1, <=512 per PSUM bank)
I_CHUNKS = [(0, 512), (512, 512), (1024, 384)]
H_CHUNKS = [(c, 512) for c in range(0, H, 512)]


# --- walrus workaround: this build rejects >1 sync wait per instruction.
# Split extras into standalone single-wait EventSemaphore instructions placed
# immediately before, on the same engine queue.
def _split_multi_waits(bir_json: bytes) -> bytes:
    j = json.loads(bir_json)
    changed = False
    for fn in j.get("functions", []):
        for blk in fn.get("blocks", []):
            out = []
            for inst in blk.get("instructions", []):
                si = inst.get("sync_info")
                waits = si.get("on_wait") if si else None
                if waits and len(waits) > 1:
                    spill, keep = waits[:-1], waits[-1:]
                    for k, w in enumerate(spill):
                        out.append({
                            "debug": inst.get("debug", 0),
                            "engine": inst["engine"],
                            "ins": [], "outs": [],
                            "name": f"{inst['name']}_w{k}",
                            "opcode": "EventSemaphore",
                            "sync_info": {"on_update": [], "on_wait": [w]},
                        })
                    si["on_wait"] = keep
                    changed = True
                out.append(inst)
            blk["instructions"] = out
    return json.dumps(j).encode() if changed else bir_json


_hook_installed = False


def _install_compile_hook():
    global _hook_installed
    if _hook_installed:
        return
    orig = bass2jax.compile_bir_kernel

    def wrapped(bir_json, tmpdir, neff_name="file.neff"):
        return orig(_split_multi_waits(bir_json), tmpdir, neff_name=neff_name)

    bass2jax.compile_bir_kernel = wrapped
    _hook_installed = True


def _cast_engine(nc, idx):
    """Round-robin the int8->bf16 weight casts across ACT/Pool/DVE.

    Balance for the engine rates (ACT 1.2G, DVE 0.96G, Pool ~0.72G effective)
    and each engine's other work: ACT 3/8, Pool 3/8, DVE 2/8.
    """
    r = idx % 8
    if r < 3:
        return nc.scalar.copy
    if r < 6:
        return nc.gpsimd.tensor_copy
    return nc.vector.tensor_copy


def _build_program(reps=1):
    nc = bass.Bass()
    x_d = nc.declare_dram_parameter("x", [T_LOC, H], F32, isOutput=False)
    w13T_d = nc.declare_dram_parameter("w13T", [E_LOC, H, 2 * I], I8, isOutput=False)
    w2T_d = nc.declare_dram_parameter("w2T", [E_LOC, I, H], I8, isOutput=False)
    wsg_d = nc.declare_dram_parameter("wsg", [E_LOC, 128, I], F32, isOutput=False)
    wsu_d = nc.declare_dram_parameter("wsu", [E_LOC, 128, I], F32, isOutput=False)
    w2s_d = nc.declare_dram_parameter("w2s", [E_LOC, 128, H], F32, isOutput=False)
    out_d = nc.declare_dram_parameter("out", [T_LOC, H], F32, isOutput=True)

    with TileContext(nc) as tc:
        with (
            tc.tile_pool(name="const", bufs=1) as const,
            tc.tile_pool(name="xload", bufs=1) as xload,
            tc.tile_pool(name="xq", bufs=1) as xqp,
            tc.tile_pool(name="xqt", bufs=3) as xqtp,
            tc.tile_pool(name="small", bufs=4) as small,
            tc.tile_pool(name="wload", bufs=2) as wload,
            tc.tile_pool(name="wcast", bufs=6) as wcast,
            tc.tile_pool(name="scales", bufs=2) as scalep,
            tc.tile_pool(name="w2scale", bufs=1) as w2scalep,
            tc.tile_pool(name="hbuf", bufs=2) as hbuf,
            tc.tile_pool(name="hq", bufs=2) as hqp,
            tc.tile_pool(name="outp", bufs=2) as outp,
            tc.tile_pool(name="pt", bufs=2, space="PSUM") as ptp,
            tc.tile_pool(name="pg", bufs=2, space="PSUM") as pgp,
            tc.tile_pool(name="pu", bufs=2, space="PSUM") as pup,
            tc.tile_pool(name="p2", bufs=2, space="PSUM") as p2p,
        ):
            env = dict(locals())
            for _rep in range(reps):
                if _rep > 0:
                    env["out_d"] = nc.dram_tensor(
                        f"out_rep{_rep}", [T_LOC, H], F32).ap()
                _emit_body(nc, tc, env)
    return nc


def _emit_body(nc, tc, pools):
    const = pools["const"]; xload = pools["xload"]; xqp = pools["xq"] if "xq" in pools else pools["xqp"]
    xqp = pools["xqp"]; xqtp = pools["xqtp"]; small = pools["small"]
    wload = pools["wload"]; wcast = pools["wcast"]; scalep = pools["scalep"]
    w2scalep = pools["w2scalep"]; hbuf = pools["hbuf"]; hqp = pools["hqp"]
    outp = pools["outp"]; ptp = pools["ptp"]; pgp = pools["pgp"]
    pup = pools["pup"]; p2p = pools["p2p"]
    x_d = pools["x_d"]; w13T_d = pools["w13T_d"]; w2T_d = pools["w2T_d"]
    wsg_d = pools["wsg_d"]; wsu_d = pools["wsu_d"]; w2s_d = pools["w2s_d"]
    out_d = pools["out_d"]

    ident = const.tile([128, 128], BF16)
    make_identity(nc, ident)

    xqT = {}     # t-block -> [128h, KT1, 128t] bf16
    s1s = {}     # t-block -> [128, 1] f32 quant scale
    cast_n = [0]

    def cast(dst, src):
        _cast_engine(nc, cast_n[0])(dst, src)
        cast_n[0] += 1

    def quantize_tb(tb):
        xt = xload.tile([128, H], F32, tag="xt")
        nc.sync.dma_start(xt[:], x_d[tb * 128:(tb + 1) * 128, :])
        amax = small.tile([128, 1], F32, tag="amax1")
        nc.vector.tensor_reduce(amax[:], xt[:], axis=AX.X, op=ALU.max,
                                apply_absolute_value=True)
        s1 = small.tile([128, 1], F32, tag="s1")
        nc.vector.tensor_scalar(s1[:], amax[:], 1.0 / 127.0, None, op0=ALU.mult)
        inv1 = small.tile([128, 1], F32, tag="inv1")
        nc.vector.reciprocal(inv1[:], s1[:])
        xq_i8 = xqp.tile([128, H], I8, tag="xq_i8")
        nc.vector.tensor_scalar(xq_i8[:], xt[:], inv1[:], None, op0=ALU.mult)
        xq_bf = xqp.tile([128, H], BF16, tag="xq_bf")
        nc.scalar.copy(xq_bf[:], xq_i8[:])
        xqt = xqtp.tile([128, KT1, 128], BF16, tag="xqT")
        # batch 8 transposes into one PSUM bank, evict with one copy
        for k0 in range(0, KT1, 8):
            pt = ptp.tile([128, 8, 128], BF16, tag="pt")
            for dk in range(8):
                k = k0 + dk
                nc.tensor.transpose(pt[:, dk, :],
                                    xq_bf[:, k * 128:(k + 1) * 128], ident[:])
            (nc.scalar.copy if (k0 // 8) % 2 else nc.vector.tensor_copy)(
                xqt[:, k0:k0 + 8, :], pt[:])
        xqT[tb] = xqt
        s1s[tb] = s1

    def mm1_chunk(e, tbs, c0, cw, wsg, wsu, hts):
        wg_i8 = [wload.tile([128, KT1 // 2, cw], I8, tag="wg_i8",
                            name=f"wg_i8_{e}_{c0}_{h2}") for h2 in range(2)]
        wu_i8 = [wload.tile([128, KT1 // 2, cw], I8, tag="wu_i8",
                            name=f"wu_i8_{e}_{c0}_{h2}") for h2 in range(2)]
        g_src = w13T_d[e, :, c0:c0 + cw].rearrange("(k p) o -> p k o", p=128)
        u_src = w13T_d[e, :, I + c0:I + c0 + cw].rearrange(
            "(k p) o -> p k o", p=128)
        for h2 in range(2):
            ksl = slice(h2 * (KT1 // 2), (h2 + 1) * (KT1 // 2))
            nc.sync.dma_start(wg_i8[h2][:], g_src[:, ksl, :])
            nc.sync.dma_start(wu_i8[h2][:], u_src[:, ksl, :])

        pg = [pgp.tile([128, cw], F32, tag="pg", name=f"pg{i}")
              for i in range(len(tbs))]
        pu = [pup.tile([128, cw], F32, tag="pu", name=f"pu{i}")
              for i in range(len(tbs))]
        QK = 4  # k-tiles per cast op
        for kq in range(KT1 // QK):
            h2, kkq = divmod(kq, (KT1 // 2) // QK)
            ks = slice(kkq * QK, (kkq + 1) * QK)
            wg_bf = wcast.tile([128, QK, cw], BF16, tag="wbf", name="wg_bf")
            cast(wg_bf[:], wg_i8[h2][:, ks, :])
            wu_bf = wcast.tile([128, QK, cw], BF16, tag="wbf", name="wu_bf")
            cast(wu_bf[:], wu_i8[h2][:, ks, :])
            for dk in range(QK):
                k = kq * QK + dk
                st, sp = (k == 0), (k == KT1 - 1)
                for i_tb, tb in enumerate(tbs):
                    nc.tensor.matmul(pg[i_tb][:], xqT[tb][:, k, :],
                                     wg_bf[:, dk, :], start=st, stop=sp)
                    nc.tensor.matmul(pu[i_tb][:], xqT[tb][:, k, :],
                                     wu_bf[:, dk, :], start=st, stop=sp)

        for i_tb, tb in enumerate(tbs):
            gate = outp.tile([128, cw], F32, tag="gate")
            nc.vector.scalar_tensor_tensor(
                gate[:], pg[i_tb][:], s1s[tb][:], wsg[:, c0:c0 + cw],
                op0=ALU.mult, op1=ALU.mult)
            up = outp.tile([128, cw], F32, tag="up")
            nc.vector.scalar_tensor_tensor(
                up[:], pu[i_tb][:], s1s[tb][:], wsu[:, c0:c0 + cw],
                op0=ALU.mult, op1=ALU.mult)
            sg = outp.tile([128, cw], F32, tag="sg")
            nc.scalar.activation(sg[:], gate[:], AF.Silu)
            nc.vector.tensor_mul(hts[i_tb][:, c0:c0 + cw], sg[:], up[:])

    def requant_tb(ht):
        amax2 = small.tile([128, 1], F32, tag="amax2")
        nc.vector.tensor_reduce(amax2[:], ht[:], axis=AX.X, op=ALU.max,
                                apply_absolute_value=True)
        s2 = small.tile([128, 1], F32, tag="s2")
        nc.vector.tensor_scalar(s2[:], amax2[:], 1.0 / 127.0, None,
                                op0=ALU.mult)
        inv2 = small.tile([128, 1], F32, tag="inv2")
        nc.vector.reciprocal(inv2[:], s2[:])
        hq_i8 = hqp.tile([128, I], I8, tag="hq_i8")
        nc.vector.tensor_scalar(hq_i8[:], ht[:], inv2[:], None, op0=ALU.mult)
        hq_bf = hqp.tile([128, I], BF16, tag="hq_bf")
        nc.scalar.copy(hq_bf[:], hq_i8[:])
        hqt = hqp.tile([128, KT2, 128], BF16, tag="hqT")
        for k0 in range(0, KT2, 8):
            kn = min(8, KT2 - k0)
            pt = ptp.tile([128, 8, 128], BF16, tag="pt", name="pt_hq")
            for dk in range(kn):
                k = k0 + dk
                nc.tensor.transpose(pt[:, dk, :],
                                    hq_bf[:, k * 128:(k + 1) * 128], ident[:])
            (nc.scalar.copy if (k0 // 8) % 2 else nc.vector.tensor_copy)(
                hqt[:, k0:k0 + kn, :], pt[:, 0:kn, :])
        return hqt, s2

    def mm2_chunk(e, tbs, c0, cw, hqT, s2s, w2s):
        w2_i8 = wload.tile([128, KT2, cw], I8, tag="w2_i8")
        nc.sync.dma_start(
            w2_i8[:],
            w2T_d[e, :, c0:c0 + cw].rearrange("(k p) o -> p k o", p=128))
        p2 = [p2p.tile([128, cw], F32, tag="p2", name=f"p2_{i}")
              for i in range(len(tbs))]
        k = 0
        for qn in (4, 4, 3):
            w2_bf = wcast.tile([128, qn, cw], BF16, tag="wbf", name="w2_bf")
            cast(w2_bf[:], w2_i8[:, k:k + qn, :])
            for dk in range(qn):
                for i_tb in range(2):
                    nc.tensor.matmul(p2[i_tb][:], hqT[i_tb][:, k, :],
                                     w2_bf[:, dk, :], start=(k == 0),
                                     stop=(k == KT2 - 1))
                k += 1
        for i_tb, tb in enumerate(tbs):
            ot = outp.tile([128, cw], F32, tag="ot", bufs=4)
            nc.vector.scalar_tensor_tensor(
                ot[:], p2[i_tb][:], s2s[i_tb][:], w2s[:, c0:c0 + cw],
                op0=ALU.mult, op1=ALU.mult)
            nc.sync.dma_start(out_d[tb * 128:(tb + 1) * 128, c0:c0 + cw],
                              ot[:])

    # ---- Expert loop ----
    for e in range(E_LOC):
        tbs = [2 * e, 2 * e + 1]
        for tb in tbs:
            quantize_tb(tb)

        wsg = scalep.tile([128, I], F32, tag="wsg")
        nc.sync.dma_start(wsg[:], wsg_d[e])
        wsu = scalep.tile([128, I], F32, tag="wsu")
        nc.sync.dma_start(wsu[:], wsu_d[e])

        hts = [hbuf.tile([128, I], F32, tag="ht", name=f"ht{e}_{i}")
               for i in range(len(tbs))]
        for (c0, cw) in I_CHUNKS:
            mm1_chunk(e, tbs, c0, cw, wsg, wsu, hts)

        hqT = []
        s2s = []
        for i_tb in range(len(tbs)):
            hqt, s2 = requant_tb(hts[i_tb])
            hqT.append(hqt)
            s2s.append(s2)

        w2s = w2scalep.tile([128, H], F32, tag="w2s")
        nc.sync.dma_start(w2s[:], w2s_d[e])
        for (c0, cw) in H_CHUNKS:
            mm2_chunk(e, tbs, c0, cw, hqT, s2s, w2s)


_cached_nc = None


def _make_in_maps(x, w13, w2, w13_scale, smooth_scale_2, w2_scale):
    x = np.asarray(x, dtype=np.float32)
    w13 = np.asarray(w13).astype(np.int8, copy=False)
    w2 = np.asarray(w2).astype(np.int8, copy=False)
    w13_scale = np.asarray(w13_scale, dtype=np.float32)
    smooth_scale_2 = np.asarray(smooth_scale_2, dtype=np.float32)
    w2_scale = np.asarray(w2_scale, dtype=np.float32)

    # Fold the (linear) smooth scale into the up-projection dequant scale.
    wsu_full = w13_scale[:, I:] * smooth_scale_2          # [E, I]
    wsg_full = w13_scale[:, :I]                           # [E, I]

    in_maps = []
    for c in range(NCORES):
        es = slice(E_LOC * c, E_LOC * (c + 1))
        ts = slice(T_LOC * c, T_LOC * (c + 1))
        in_maps.append({
            "x": np.ascontiguousarray(x[ts]),
            "w13T": np.ascontiguousarray(w13[es].transpose(0, 2, 1)),
            "w2T": np.ascontiguousarray(w2[es].transpose(0, 2, 1)),
            "wsg": np.ascontiguousarray(
                np.broadcast_to(wsg_full[es, None, :], (E_LOC, 128, I))),
            "wsu": np.ascontiguousarray(
                np.broadcast_to(wsu_full[es, None, :], (E_LOC, 128, I))),
            "w2s": np.ascontiguousarray(
                np.broadcast_to(w2_scale[es, None, :], (E_LOC, 128, H))),
        })
    return in_maps


def _run(in_maps, **kwargs):
    global _cached_nc
    _install_compile_hook()
    if _cached_nc is None:
        _cached_nc = _build_program()
    return run_bass_kernel_spmd(_cached_nc, in_maps, list(range(NCORES)),
                                **kwargs)


def kernel(x, w13, w2, w13_scale, smooth_scale_2, w2_scale, expert_tokens):
    # expert_tokens describes the fixed equal contiguous grouping (the
    # reference ignores it); we rely on that same grouping.
    del expert_tokens
    in_maps = _make_in_maps(x, w13, w2, w13_scale, smooth_scale_2, w2_scale)
    res = _run(in_maps)
    return np.concatenate([res.results[c]["out"] for c in range(NCORES)],
                          axis=0)


def run_profiled(x, w13, w2, w13_scale, smooth_scale_2, w2_scale,
                 expert_tokens):
    """test.py helper: run with NTFF profiling, return BassKernelResults."""
    del expert_tokens
    in_maps = _make_in_maps(x, w13, w2, w13_scale, smooth_scale_2, w2_scale)
    return _run(in_maps, trace=True)
